# revision 10
# baseline (speedup 1.0000x reference)
"""Trainium2 Bass kernel for nn_BinaryMemory (retrieval_knn).

reference:
    gated = sigmoid(query @ W.T + b)                      # [1, D], D=4096
    sims  = 1 - mean(|memory - gated|, axis=-1)           # [N],   N=16384
    mask  = sims >= 0.8

Sharding (8 cores, no collectives): shard the D axis; core c owns
d-chunk [c*512, (c+1)*512). All bulk tensors stream as fp8_e3m4.
Layout is d-on-partitions (memory shard transposed host-side to
[512 d, 16384 n]) so the gate value g[d] is a per-partition scalar.

|m-g| split: DVE pieces compute min(m-g,0) via one fused
tensor_scalar(sub, min); the m-term sum rides on the PE (ones^T @ m,
gate-independent) and the g-term is corrected on host per (k, group).
ScalarE pieces compute |m-g| in one op via activation(Abs, scale=-1,
bias=+g). Reductions over d run on the PE into psum rows at quadrant
offsets (4-way tile_position concurrency).

v5 design, driven by traces of 4 prior variants:
 - The single HWDGE stream delivers COMPLETIONS as a FIFO conveyor at
   ~1.25us per DMA nearly independent of size (HBM-receipt pipeline),
   so DMA COUNT is the currency: W ships as ONE 2MB DMA, the 16 memory
   tiles are unsplit, and outputs are 5 DMAs ([4,1024] fp16 per k + the
   gate-sum row). No SWDGE anywhere (it skews SDMA engine 15 ~10%
   slower from t0); no DMA gen on the ScalarE queue (it costs ABS
   throughput); outputs ride the sync ring after the last input gen.
 - The gate runs per 128-d chunk (32 matmuls each, quadrant-cycled;
   fused strip-sum/transpose matmul; Sigmoid straight off psum with
   per-partition bias b), with the PE HAM-warmed by junk matmuls so
   g[c] lands at ~0.9us intervals right behind the single W completion
   - just ahead of the first four tiles' completions. Elementwise
   starts ~5us earlier than with the monolithic gate.
 - psum->SBUF bank copies are fp16 (2x engine rate, half the out-DMA
   bytes); the host upcasts.
"""
import sys

sys.path.insert(0, "/opt/trn_rl_repo")

import numpy as np
import ml_dtypes

import concourse.bacc as bacc
import concourse.mybir as mybir
import concourse.tile as tile
from concourse.bass_utils import run_bass_kernel_spmd

N_CORES = 8
D = 4096
N = 16384
D_SH = D // N_CORES            # 512 dims per core
DC = D_SH // 128               # 4 d-chunks (partition blocks)
NT = 4096                      # n per tile
NK = N // NT                   # 4 n-chunks
NG = NT // 512                 # 8 psum groups per tile
THRESHOLD = 0.8

# ScalarE (Abs) tiles; everything else takes the DVE min path
_ACT_TILES = {(0, 3), (1, 2), (2, 0), (2, 2), (3, 0)}


def _dve_chunks(k, j):
    return [c for c in range(DC) if (k, c) not in _ACT_TILES]


_CACHE = {}


def _build():
    f32 = mybir.dt.float32
    f16 = mybir.dt.float16
    f8 = mybir.dt.float8e3
    A = mybir.AluOpType
    AF = mybir.ActivationFunctionType
    nc = bacc.Bacc(
        "TRN2", target_bir_lowering=False, debug=False, num_devices=N_CORES
    )

    memT = nc.dram_tensor("memT", [D_SH, N], f8, kind="ExternalInput")
    # W shard, host-packed c-major:
    # wtpc[p, c*4096 + j*128 + n'] = W[cglobal*128 + n', j*128 + p]
    wtpc = nc.dram_tensor("wtpc", [128, 32 * D_SH], f8, kind="ExternalInput")
    # packed fp8 constants: cols 0:32 qcol, 32 ones, 33 neg2, 34 selsum
    c8 = nc.dram_tensor("c8", [128, 35], f8, kind="ExternalInput")
    # packed f32 constants: cols 0:4 b columns, 4 ones
    c32 = nc.dram_tensor("c32", [128, 5], f32, kind="ExternalInput")
    # row i, col k*1024 + 512h + n: group j=4h+i of block k
    outp = nc.dram_tensor("outp", [4, 4096], f16, kind="ExternalOutput")
    outg = nc.dram_tensor("outg", [1, DC], f16, kind="ExternalOutput")

    with tile.TileContext(nc) as tc:
        with (
            tc.tile_pool(name="wts", bufs=1) as wpool,
            tc.tile_pool(name="mem", bufs=16) as mpool,
            tc.tile_pool(name="dts", bufs=3) as dpool,
            tc.tile_pool(name="acts", bufs=2) as apool,
            tc.tile_pool(name="cp", bufs=1) as cppool,
            tc.tile_pool(name="small", bufs=1) as spool,
            tc.tile_pool(name="psz", bufs=1, space="PSUM") as pzpool,
            tc.tile_pool(name="pst", bufs=1, space="PSUM") as ptpool,
            tc.tile_pool(name="psb", bufs=6, space="PSUM") as pspool,
        ):
            # ---- constants first, then the single W DMA ----
            c8_sb = spool.tile([128, 35], f8, tag="c8")
            nc.sync.dma_start(out=c8_sb[:], in_=c8[:])
            wt_a = wpool.tile([128, 16 * D_SH], f8, tag="wta")
            nc.sync.dma_start(out=wt_a[:], in_=wtpc[:, 0 : 16 * D_SH])
            mt00 = mpool.tile([128, NT], f8, tag="m", name="mt00")
            nc.sync.dma_start(out=mt00[:], in_=memT[0:128, 0:NT])
            wt_b = wpool.tile([128, 16 * D_SH], f8, tag="wtb")
            nc.sync.dma_start(out=wt_b[:], in_=wtpc[:, 16 * D_SH : 32 * D_SH])
            c32_sb = spool.tile([128, 5], f32, tag="c32")
            nc.scalar.dma_start(out=c32_sb[:], in_=c32[:])
            qc_sb = c8_sb[:, 0:32]
            ones_sb = c8_sb[:, 32:33]
            neg2_sb = c8_sb[:, 33:34]
            selsum8 = c8_sb[:, 34:35]
            b4 = c32_sb[:, 0:4]
            ones32_sb = c32_sb[:, 4:5]
            # activation table preload with no DMA dependency
            dum_in = spool.tile([1, 4], f32, tag="dumin")
            nc.vector.memset(dum_in[:], 0.25)
            dum = spool.tile([1, 4], f32, tag="dum")
            nc.scalar.activation(dum[:], dum_in[:], AF.Sigmoid)
            nc.scalar.activation(dum[:], dum_in[:], AF.Abs)

            zps = pzpool.tile([128, D_SH], f32, tag="z")
            nc.vector.memset(zps[:], 0.0)
            ztp = ptpool.tile([128, 512], f32, tag="ztp")
            zcp = spool.tile([128, D_SH], f16, tag="zcp")
            gpos = spool.tile([128, DC], f32, tag="gpos")

            # PE warmup: junk matmuls (zeroed SBUF, psum region reused by
            # strip-sums whose start=True clears it) keep HAM at K=8/8
            # through the W-completion so the gate runs at 2.4GHz.
            wjunk = spool.tile([128, 512], f8, tag="wjunk")
            nc.vector.memset(wjunk[:], 0.5)
            for _w in range(14):
                nc.tensor.matmul(
                    ztp[0:1, :], wjunk[:, 0:1], wjunk[:, 0:512],
                    start=(_w == 0), stop=(_w == 13), skip_group_check=True,
                )

            # ---- gate, per 128-d chunk ----
            for c in range(DC):
                for j in range(32):
                    r = j % 4
                    nc.tensor.matmul(
                        zps[32 * r : 32 * r + 1, c * 128 : (c + 1) * 128],
                        qc_sb[:, j : j + 1],
                        (wt_a if c < 2 else wt_b)[
                            :, (c % 2) * NT + j * 128 : (c % 2) * NT + (j + 1) * 128
                        ],
                        start=(j < 4),
                        stop=(j >= 28),
                        tile_position=(0, 32 * r),
                        skip_group_check=True,
                    )
                with tc.high_priority():
                    csl = slice(c * 128, (c + 1) * 128)
                    nc.scalar.activation(zcp[:, csl], zps[:, csl], AF.Copy)
                    # fused strip-sum + transpose
                    nc.tensor.matmul(
                        ztp[:, c : c + 1],
                        zcp[:, csl],
                        selsum8,
                        start=True,
                        stop=True,
                        skip_group_check=True,
                    )
                    # g_c = sigmoid(z_c + b_c) straight from psum
                    nc.scalar.activation(
                        gpos[:, c : c + 1], ztp[:, c : c + 1], AF.Sigmoid,
                        bias=b4[:, c : c + 1],
                    )

            # gate-sum correction, right after the gate
            with tc.high_priority():
                gsps = ztp[0:1, 4:8]
                nc.tensor.matmul(
                    gsps, ones32_sb, gpos[:], start=True, stop=True,
                    skip_group_check=True,
                )
                gs_sb = spool.tile([1, DC], f16, tag="gs")
                nc.scalar.activation(gs_sb[:], gsps, AF.Copy)

            nc.scalar.dma_start(out=outg[:], in_=gs_sb[:])

            # ---- main loop ----
            cpm = cppool.tile([128, 4 * 1024], f16, tag="cpm")
            for k in range(NK):
                banks = [
                    pspool.tile([128, 512], f32, tag="bank", name=f"bank{k}a"),
                    pspool.tile([128, 512], f32, tag="bank", name=f"bank{k}b"),
                ]
                total = [0] * NG
                for c in range(DC):
                    npass = 1 if (k, c) in _ACT_TILES else 2
                    for j in range(NG):
                        total[j] += npass
                seen = [0] * NG
                mts = []
                for c in range(DC):
                    if k == 0 and c == 0:
                        mt = mt00
                    else:
                        mt = mpool.tile([128, NT], f8, tag="m")
                        nc.sync.dma_start(
                            out=mt[:],
                            in_=memT[c * 128 : (c + 1) * 128, k * NT : (k + 1) * NT],
                        )
                    mts.append(mt)
                    if (k, c) not in _ACT_TILES:
                        for j in range(NG):
                            nc.tensor.matmul(
                                banks[j // 4][32 * (j % 4) : 32 * (j % 4) + 1, :],
                                ones_sb,
                                mt[:, j * 512 : (j + 1) * 512],
                                start=(seen[j] == 0),
                                stop=(seen[j] == total[j] - 1),
                                tile_position=(0, 32 * (j % 4)),
                                skip_group_check=True,
                            )
                            seen[j] += 1
                for c in range(DC):
                    mt = mts[c]
                    halves = 2 if (k, c) == (3, 3) else 1
                    if (k, c) in _ACT_TILES:
                        at = apool.tile([128, NT], f8, tag="a")
                        src_, stat = at, ones_sb
                    else:
                        dt = dpool.tile([128, NT], f8, tag="d")
                        src_, stat = dt, neg2_sb
                    for v in range(halves):
                        sl_v = slice(v * NT // halves, (v + 1) * NT // halves)
                        if (k, c) in _ACT_TILES:
                            nc.scalar.activation(
                                at[:, sl_v], mt[:, sl_v], AF.Abs,
                                bias=gpos[:, c : c + 1], scale=-1.0,
                            )
                        else:
                            nc.vector.tensor_scalar(
                                dt[:, sl_v], mt[:, sl_v],
                                gpos[:, c : c + 1], 0.0,
                                A.subtract, A.min,
                            )
                        for j in range(
                            v * NG // halves, (v + 1) * NG // halves
                        ):
                            nc.tensor.matmul(
                                banks[j // 4][32 * (j % 4) : 32 * (j % 4) + 1, :],
                                stat,
                                src_[:, j * 512 : (j + 1) * 512],
                                start=(seen[j] == 0),
                                stop=(seen[j] == total[j] - 1),
                                tile_position=(0, 32 * (j % 4)),
                                skip_group_check=True,
                            )
                            seen[j] += 1
                    # PE keep-warm filler (HAM stays at K=8/8 so reduce
                    # passes run at 2.4GHz; zps is dead after the gate)
                    nc.tensor.matmul(
                        zps[0:1, :], wjunk[:, 0:1], wjunk[:, 0:512],
                        start=True, stop=True, skip_group_check=True,
                    )
                for h in range(2):
                    csl = slice(1024 * k + 512 * h, 1024 * k + 512 * h + 512)
                    if k == 3 and h == 1:
                        nc.vector.tensor_copy(cpm[:, csl], banks[h][:])
                    else:
                        nc.scalar.activation(cpm[:, csl], banks[h][:], AF.Copy)

            # ---- single result DMA on the (empty) scalar ring ----
            nc.scalar.dma_start(out=outp[:], in_=cpm[0:128:32, :])

    nc.compile()
    return nc


def _get_nc():
    if "nc" not in _CACHE:
        _CACHE["nc"] = _build()
    return _CACHE["nc"]


def kernel(query, W, b, memory, _trace=False, _return_raw=False):
    f8 = ml_dtypes.float8_e3m4
    query = np.asarray(query, dtype=np.float32)
    W = np.asarray(W, dtype=np.float32)
    b = np.asarray(b, dtype=np.float32)
    memory = np.asarray(memory, dtype=np.float32)

    mem8T = np.ascontiguousarray(memory.astype(f8).T)       # [D, N] fp8
    W8 = W.astype(f8)
    q8 = query.reshape(32, 128).astype(f8).T                # [128, 32]
    c8 = np.zeros((128, 35), dtype=f8)
    c8[:, 0:32] = q8
    c8[:, 32] = f8(1.0)
    c8[:, 33] = f8(-2.0)
    c8[0:128:32, 34] = f8(1.0)

    in_maps = []
    for c in range(N_CORES):
        sl = slice(c * D_SH, (c + 1) * D_SH)
        wsh = W8[sl, :]                       # [512, 4096]
        # wtpc[p, cc*4096 + j*128 + n'] = wsh[cc*128 + n', j*128 + p]
        wtpc = np.ascontiguousarray(
            wsh.reshape(4, 128, 32, 128).transpose(3, 0, 2, 1).reshape(128, -1)
        )
        c32 = np.zeros((128, 5), dtype=np.float32)
        c32[:, 0:4] = b[sl].reshape(4, 128).T
        c32[:, 4] = 1.0
        in_maps.append(
            {
                "memT": np.ascontiguousarray(mem8T[sl, :]),
                "wtpc": wtpc,
                "c8": c8,
                "c32": c32,
            }
        )

    nc = _get_nc()
    res = run_bass_kernel_spmd(
        nc, in_maps, list(range(N_CORES)), trace=_trace
    )

    total = np.zeros(N, dtype=np.float64)
    for c in range(N_CORES):
        out = res.results[c]["outp"].astype(np.float64)
        gsum = res.results[c]["outg"].astype(np.float64)[0]
        # row i, col k*1024+512h+n  ->  block k, group j=4h+i
        rows = out.reshape(4, NK, 2, 512).transpose(1, 2, 0, 3)
        rows = np.ascontiguousarray(rows).reshape(NK, NG, 512)
        corr = np.array(
            [
                [sum(gsum[ci] for ci in _dve_chunks(k, j)) for j in range(NG)]
                for k in range(NK)
            ]
        )
        total += (rows - corr[:, :, None]).reshape(N)
    sims = (1.0 - total / D).astype(np.float32)
    mask = sims >= THRESHOLD
    if _return_raw:
        return (sims, mask), res
    return sims, mask


# revision 11
# speedup vs baseline: 1.0060x; 1.0060x over previous
"""Trainium2 Bass kernel for nn_BinaryMemory (retrieval_knn).

reference:
    gated = sigmoid(query @ W.T + b)                      # [1, D], D=4096
    sims  = 1 - mean(|memory - gated|, axis=-1)           # [N],   N=16384
    mask  = sims >= 0.8

Sharding (8 cores, no collectives): shard the D axis; core c owns
d-chunk [c*512, (c+1)*512). All bulk tensors stream as fp8_e3m4.
Layout is d-on-partitions (memory shard transposed host-side to
[512 d, 16384 n]) so the gate value g[d] is a per-partition scalar.

|m-g| split: DVE pieces compute min(m-g,0) via one fused
tensor_scalar(sub, min); the m-term sum rides on the PE (ones^T @ m,
gate-independent) and the g-term is corrected on host per (k, group).
ScalarE pieces compute |m-g| in one op via activation(Abs, scale=-1,
bias=+g). Reductions over d run on the PE into psum rows at quadrant
offsets (4-way tile_position concurrency).

v5 design, driven by traces of 4 prior variants:
 - The single HWDGE stream delivers COMPLETIONS as a FIFO conveyor at
   ~1.25us per DMA nearly independent of size (HBM-receipt pipeline),
   so DMA COUNT is the currency: W ships as ONE 2MB DMA, the 16 memory
   tiles are unsplit, and outputs are 5 DMAs ([4,1024] fp16 per k + the
   gate-sum row). No SWDGE anywhere (it skews SDMA engine 15 ~10%
   slower from t0); no DMA gen on the ScalarE queue (it costs ABS
   throughput); outputs ride the sync ring after the last input gen.
 - The gate runs per 128-d chunk (32 matmuls each, quadrant-cycled;
   fused strip-sum/transpose matmul; Sigmoid straight off psum with
   per-partition bias b), with the PE HAM-warmed by junk matmuls so
   g[c] lands at ~0.9us intervals right behind the single W completion
   - just ahead of the first four tiles' completions. Elementwise
   starts ~5us earlier than with the monolithic gate.
 - psum->SBUF bank copies are fp16 (2x engine rate, half the out-DMA
   bytes); the host upcasts.
"""
import sys

sys.path.insert(0, "/opt/trn_rl_repo")

import numpy as np
import ml_dtypes

import concourse.bacc as bacc
import concourse.mybir as mybir
import concourse.tile as tile
from concourse.bass_utils import run_bass_kernel_spmd

N_CORES = 8
D = 4096
N = 16384
D_SH = D // N_CORES            # 512 dims per core
DC = D_SH // 128               # 4 d-chunks (partition blocks)
NT = 4096                      # n per tile
NK = N // NT                   # 4 n-chunks
NG = NT // 512                 # 8 psum groups per tile
THRESHOLD = 0.8

# ScalarE (Abs) tiles; everything else takes the DVE min path
_ACT_TILES = {(0, 3), (1, 2), (2, 0), (2, 2), (3, 0)}


def _dve_chunks(k, j):
    return [c for c in range(DC) if (k, c) not in _ACT_TILES]


_CACHE = {}


def _build():
    f32 = mybir.dt.float32
    f16 = mybir.dt.float16
    f8 = mybir.dt.float8e3
    A = mybir.AluOpType
    AF = mybir.ActivationFunctionType
    nc = bacc.Bacc(
        "TRN2", target_bir_lowering=False, debug=False, num_devices=N_CORES
    )

    memT = nc.dram_tensor("memT", [D_SH, N], f8, kind="ExternalInput")
    # W shard, host-packed c-major:
    # wtpc[p, c*4096 + j*128 + n'] = W[cglobal*128 + n', j*128 + p]
    wtpc = nc.dram_tensor("wtpc", [128, 32 * D_SH], f8, kind="ExternalInput")
    # packed fp8 constants: cols 0:32 qcol, 32 ones, 33 neg2, 34 selsum
    c8 = nc.dram_tensor("c8", [128, 35], f8, kind="ExternalInput")
    # packed f32 constants: cols 0:4 b columns, 4 ones
    c32 = nc.dram_tensor("c32", [128, 5], f32, kind="ExternalInput")
    # row i, col k*1024 + 512h + n: group j=4h+i of block k
    outp = nc.dram_tensor("outp", [4, 4096], f16, kind="ExternalOutput")
    outg = nc.dram_tensor("outg", [1, DC], f16, kind="ExternalOutput")

    with tile.TileContext(nc) as tc:
        with (
            tc.tile_pool(name="wts", bufs=1) as wpool,
            tc.tile_pool(name="mem", bufs=16) as mpool,
            tc.tile_pool(name="dts", bufs=3) as dpool,
            tc.tile_pool(name="acts", bufs=2) as apool,
            tc.tile_pool(name="cp", bufs=1) as cppool,
            tc.tile_pool(name="small", bufs=1) as spool,
            tc.tile_pool(name="psz", bufs=1, space="PSUM") as pzpool,
            tc.tile_pool(name="pst", bufs=1, space="PSUM") as ptpool,
            tc.tile_pool(name="psb", bufs=6, space="PSUM") as pspool,
        ):
            # ---- constants first, then the single W DMA ----
            c8_sb = spool.tile([128, 35], f8, tag="c8")
            nc.sync.dma_start(out=c8_sb[:], in_=c8[:])
            wt_a = wpool.tile([128, 16 * D_SH], f8, tag="wta")
            nc.sync.dma_start(out=wt_a[:], in_=wtpc[:, 0 : 16 * D_SH])
            mt00 = mpool.tile([128, NT], f8, tag="m", name="mt00")
            nc.sync.dma_start(out=mt00[:], in_=memT[0:128, 0:NT])
            wt_b = wpool.tile([128, 16 * D_SH], f8, tag="wtb")
            nc.sync.dma_start(out=wt_b[:], in_=wtpc[:, 16 * D_SH : 32 * D_SH])
            c32_sb = spool.tile([128, 5], f32, tag="c32")
            nc.scalar.dma_start(out=c32_sb[:], in_=c32[:])
            qc_sb = c8_sb[:, 0:32]
            ones_sb = c8_sb[:, 32:33]
            neg2_sb = c8_sb[:, 33:34]
            selsum8 = c8_sb[:, 34:35]
            b4 = c32_sb[:, 0:4]
            ones32_sb = c32_sb[:, 4:5]
            # activation table preload with no DMA dependency
            dum_in = spool.tile([1, 4], f32, tag="dumin")
            nc.vector.memset(dum_in[:], 0.25)
            dum = spool.tile([1, 4], f32, tag="dum")
            nc.scalar.activation(dum[:], dum_in[:], AF.Sigmoid)
            nc.scalar.activation(dum[:], dum_in[:], AF.Abs)

            zps = pzpool.tile([128, D_SH], f32, tag="z")
            nc.vector.memset(zps[:], 0.0)
            ztp = ptpool.tile([128, 512], f32, tag="ztp")
            zcp = spool.tile([128, D_SH], f16, tag="zcp")
            gpos = spool.tile([128, DC], f32, tag="gpos")

            # PE warmup: junk matmuls (zeroed SBUF, psum region reused by
            # strip-sums whose start=True clears it) keep HAM at K=8/8
            # through the W-completion so the gate runs at 2.4GHz.
            wjunk = spool.tile([128, 512], f8, tag="wjunk")
            nc.vector.memset(wjunk[:], 0.5)
            for _w in range(14):
                nc.tensor.matmul(
                    ztp[0:1, :], wjunk[:, 0:1], wjunk[:, 0:512],
                    start=(_w == 0), stop=(_w == 13), skip_group_check=True,
                )

            # ---- gate, per 128-d chunk ----
            for c in range(DC):
                for j in range(32):
                    r = j % 4
                    nc.tensor.matmul(
                        zps[32 * r : 32 * r + 1, c * 128 : (c + 1) * 128],
                        qc_sb[:, j : j + 1],
                        (wt_a if c < 2 else wt_b)[
                            :, (c % 2) * NT + j * 128 : (c % 2) * NT + (j + 1) * 128
                        ],
                        start=(j < 4),
                        stop=(j >= 28),
                        tile_position=(0, 32 * r),
                        skip_group_check=True,
                    )
                with tc.high_priority():
                    csl = slice(c * 128, (c + 1) * 128)
                    nc.scalar.activation(zcp[:, csl], zps[:, csl], AF.Copy)
                    # fused strip-sum + transpose
                    nc.tensor.matmul(
                        ztp[:, c : c + 1],
                        zcp[:, csl],
                        selsum8,
                        start=True,
                        stop=True,
                        skip_group_check=True,
                    )
                    # g_c = sigmoid(z_c + b_c) straight from psum
                    nc.scalar.activation(
                        gpos[:, c : c + 1], ztp[:, c : c + 1], AF.Sigmoid,
                        bias=b4[:, c : c + 1],
                    )

            # gate-sum correction, right after the gate
            with tc.high_priority():
                gsps = ztp[0:1, 4:8]
                nc.tensor.matmul(
                    gsps, ones32_sb, gpos[:], start=True, stop=True,
                    skip_group_check=True,
                )
                gs_sb = spool.tile([1, DC], f16, tag="gs")
                nc.scalar.activation(gs_sb[:], gsps, AF.Copy)

            deferred_outs = []

            # ---- main loop ----
            cpm = cppool.tile([128, 4 * 1024], f16, tag="cpm")
            for k in range(NK):
                banks = [
                    pspool.tile([128, 512], f32, tag="bank", name=f"bank{k}a"),
                    pspool.tile([128, 512], f32, tag="bank", name=f"bank{k}b"),
                ]
                total = [0] * NG
                for c in range(DC):
                    npass = 1 if (k, c) in _ACT_TILES else 2
                    for j in range(NG):
                        total[j] += npass
                seen = [0] * NG
                mts = []
                for c in range(DC):
                    if k == 0 and c == 0:
                        mt = mt00
                    else:
                        mt = mpool.tile([128, NT], f8, tag="m")
                        nc.sync.dma_start(
                            out=mt[:],
                            in_=memT[c * 128 : (c + 1) * 128, k * NT : (k + 1) * NT],
                        )
                    mts.append(mt)
                    if (k, c) not in _ACT_TILES:
                        for j in range(NG):
                            nc.tensor.matmul(
                                banks[j // 4][32 * (j % 4) : 32 * (j % 4) + 1, :],
                                ones_sb,
                                mt[:, j * 512 : (j + 1) * 512],
                                start=(seen[j] == 0),
                                stop=(seen[j] == total[j] - 1),
                                tile_position=(0, 32 * (j % 4)),
                                skip_group_check=True,
                            )
                            seen[j] += 1
                for c in range(DC):
                    mt = mts[c]
                    halves = 2 if (k, c) == (3, 3) else 1
                    if (k, c) in _ACT_TILES:
                        at = apool.tile([128, NT], f8, tag="a")
                        src_, stat = at, ones_sb
                    else:
                        dt = dpool.tile([128, NT], f8, tag="d")
                        src_, stat = dt, neg2_sb
                    for v in range(halves):
                        sl_v = slice(v * NT // halves, (v + 1) * NT // halves)
                        if (k, c) in _ACT_TILES:
                            nc.scalar.activation(
                                at[:, sl_v], mt[:, sl_v], AF.Abs,
                                bias=gpos[:, c : c + 1], scale=-1.0,
                            )
                        else:
                            nc.vector.tensor_scalar(
                                dt[:, sl_v], mt[:, sl_v],
                                gpos[:, c : c + 1], 0.0,
                                A.subtract, A.min,
                            )
                        for j in range(
                            v * NG // halves, (v + 1) * NG // halves
                        ):
                            nc.tensor.matmul(
                                banks[j // 4][32 * (j % 4) : 32 * (j % 4) + 1, :],
                                stat,
                                src_[:, j * 512 : (j + 1) * 512],
                                start=(seen[j] == 0),
                                stop=(seen[j] == total[j] - 1),
                                tile_position=(0, 32 * (j % 4)),
                                skip_group_check=True,
                            )
                            seen[j] += 1
                    # PE keep-warm filler (HAM stays at K=8/8 so reduce
                    # passes run at 2.4GHz; zps is dead after the gate)
                    nc.tensor.matmul(
                        zps[0:1, 0:256], wjunk[:, 0:1], wjunk[:, 0:256],
                        start=True, stop=True, skip_group_check=True,
                    )
                for h in range(2):
                    csl = slice(1024 * k + 512 * h, 1024 * k + 512 * h + 512)
                    if h == 1:
                        nc.vector.tensor_copy(cpm[:, csl], banks[h][:])
                    else:
                        nc.scalar.activation(cpm[:, csl], banks[h][:], AF.Copy)

            # ---- result DMAs on the sync ring, after all input gens ----
            nc.sync.dma_start(out=outg[:], in_=gs_sb[:])
            nc.sync.dma_start(out=outp[:], in_=cpm[0:128:32, :])

    nc.compile()
    return nc


def _get_nc():
    if "nc" not in _CACHE:
        _CACHE["nc"] = _build()
    return _CACHE["nc"]


def kernel(query, W, b, memory, _trace=False, _return_raw=False):
    f8 = ml_dtypes.float8_e3m4
    query = np.asarray(query, dtype=np.float32)
    W = np.asarray(W, dtype=np.float32)
    b = np.asarray(b, dtype=np.float32)
    memory = np.asarray(memory, dtype=np.float32)

    mem8T = np.ascontiguousarray(memory.astype(f8).T)       # [D, N] fp8
    W8 = W.astype(f8)
    q8 = query.reshape(32, 128).astype(f8).T                # [128, 32]
    c8 = np.zeros((128, 35), dtype=f8)
    c8[:, 0:32] = q8
    c8[:, 32] = f8(1.0)
    c8[:, 33] = f8(-2.0)
    c8[0:128:32, 34] = f8(1.0)

    in_maps = []
    for c in range(N_CORES):
        sl = slice(c * D_SH, (c + 1) * D_SH)
        wsh = W8[sl, :]                       # [512, 4096]
        # wtpc[p, cc*4096 + j*128 + n'] = wsh[cc*128 + n', j*128 + p]
        wtpc = np.ascontiguousarray(
            wsh.reshape(4, 128, 32, 128).transpose(3, 0, 2, 1).reshape(128, -1)
        )
        c32 = np.zeros((128, 5), dtype=np.float32)
        c32[:, 0:4] = b[sl].reshape(4, 128).T
        c32[:, 4] = 1.0
        in_maps.append(
            {
                "memT": np.ascontiguousarray(mem8T[sl, :]),
                "wtpc": wtpc,
                "c8": c8,
                "c32": c32,
            }
        )

    nc = _get_nc()
    res = run_bass_kernel_spmd(
        nc, in_maps, list(range(N_CORES)), trace=_trace
    )

    total = np.zeros(N, dtype=np.float64)
    for c in range(N_CORES):
        out = res.results[c]["outp"].astype(np.float64)
        gsum = res.results[c]["outg"].astype(np.float64)[0]
        # row i, col k*1024+512h+n  ->  block k, group j=4h+i
        rows = out.reshape(4, NK, 2, 512).transpose(1, 2, 0, 3)
        rows = np.ascontiguousarray(rows).reshape(NK, NG, 512)
        corr = np.array(
            [
                [sum(gsum[ci] for ci in _dve_chunks(k, j)) for j in range(NG)]
                for k in range(NK)
            ]
        )
        total += (rows - corr[:, :, None]).reshape(N)
    sims = (1.0 - total / D).astype(np.float32)
    mask = sims >= THRESHOLD
    if _return_raw:
        return (sims, mask), res
    return sims, mask


# revision 12
# speedup vs baseline: 1.0094x; 1.0034x over previous
"""Trainium2 Bass kernel for nn_BinaryMemory (retrieval_knn).

reference:
    gated = sigmoid(query @ W.T + b)                      # [1, D], D=4096
    sims  = 1 - mean(|memory - gated|, axis=-1)           # [N],   N=16384
    mask  = sims >= 0.8

Sharding (8 cores, no collectives): shard the D axis; core c owns
d-chunk [c*512, (c+1)*512). All bulk tensors stream as fp8_e3m4.
Layout is d-on-partitions (memory shard transposed host-side to
[512 d, 16384 n]) so the gate value g[d] is a per-partition scalar.

|m-g| split: DVE tiles compute min(m-g,0) via one fused
tensor_scalar(sub, min); the m-term sum rides on the PE (ones^T @ m,
gate-independent) and the g-term is corrected on host per (k, group).
ScalarE tiles compute |m-g| in one op via activation(Abs, scale=-1,
bias=+g). Reductions over d run on the PE into psum rows at quadrant
offsets (4-way tile_position concurrency).

Design facts measured across 8 trace iterations on this stack:
 - The single HWDGE stream delivers COMPLETIONS as a FIFO conveyor at
   ~1.2-1.4us per DMA nearly independent of size (HBM-receipt
   pipeline), so DMA COUNT is the currency. W ships as TWO 1MB halves
   (c-major) with memory tile (0,0) between them: tile (0,0) becomes
   consumable at ~16us while W-half-2 still streams. The 16 memory
   tiles are single DMAs; mem pool bufs=16 so descriptor generation
   NEVER waits on the elementwise engines (buffer recycling otherwise
   couples the engine backlog into the DMA latency loop, stretching
   the stream by several us).
 - No SWDGE anywhere (gpsimd-ring DMAs skew SDMA engine 15 ~10% slower
   from t0, adding ~3us to late completions); no DMA gen on the
   ScalarE queue mid-kernel (costs ABS throughput). Outputs ride the
   sync ring, emitted after the last input gen.
 - The gate runs per 128-d chunk: 32 matmuls (quadrant-cycled), a
   fused strip-sum+transpose matmul (stationary = psum-copied z
   strips as fp16, moving = selsum column), then Sigmoid straight off
   psum with per-partition bias b. The PE is HAM-warmed by junk
   matmuls so the gate runs at 2.4GHz. g[chunk0] lands ~15.8us,
   right at tile (0,0)'s completion; later chunks stay ahead of
   their first consumer tiles.
 - psum->SBUF bank copies are fp16 (halves the out bytes; psum source
   caps the copy rate anyway); the host upcasts. Engine balance:
   11 DVE min-tiles + 5 ScalarE Abs-tiles + 7 ScalarE copies + 1 DVE
   copy saturates both engines 22->43.5us; the stream feeds them at
   service rate the whole way (measured: both >95% busy mid-kernel).
"""
import sys

sys.path.insert(0, "/opt/trn_rl_repo")

import numpy as np
import ml_dtypes

import concourse.bacc as bacc
import concourse.mybir as mybir
import concourse.tile as tile
from concourse.bass_utils import run_bass_kernel_spmd

N_CORES = 8
D = 4096
N = 16384
D_SH = D // N_CORES            # 512 dims per core
DC = D_SH // 128               # 4 d-chunks (partition blocks)
NT = 4096                      # n per tile
NK = N // NT                   # 4 n-chunks
NG = NT // 512                 # 8 psum groups per tile
THRESHOLD = 0.8

# ScalarE (Abs) tiles; everything else takes the DVE min path
_ACT_TILES = {(0, 3), (1, 2), (2, 0), (2, 2), (3, 0)}


def _dve_chunks(k, j):
    return [c for c in range(DC) if (k, c) not in _ACT_TILES]


_CACHE = {}


def _build():
    f32 = mybir.dt.float32
    f16 = mybir.dt.float16
    f8 = mybir.dt.float8e3
    A = mybir.AluOpType
    AF = mybir.ActivationFunctionType
    nc = bacc.Bacc(
        "TRN2", target_bir_lowering=False, debug=False, num_devices=N_CORES
    )

    memT = nc.dram_tensor("memT", [D_SH, N], f8, kind="ExternalInput")
    # W shard, host-packed c-major:
    # wtpc[p, c*4096 + j*128 + n'] = W[cglobal*128 + n', j*128 + p]
    wtpc = nc.dram_tensor("wtpc", [128, 32 * D_SH], f8, kind="ExternalInput")
    # packed fp8 constants: cols 0:32 qcol, 32 ones, 33 neg2, 34 selsum
    c8 = nc.dram_tensor("c8", [128, 35], f8, kind="ExternalInput")
    # packed f32 constants: cols 0:4 b columns, 4 ones
    c32 = nc.dram_tensor("c32", [128, 5], f32, kind="ExternalInput")
    # rows 4k+i: group j=4h+i of block k at cols [512h:512h+512]; row 16
    # cols 0:4 = per-chunk gate sums
    outp = nc.dram_tensor("outp", [17, 1024], f16, kind="ExternalOutput")

    with tile.TileContext(nc) as tc:
        with (
            tc.tile_pool(name="wts", bufs=1) as wpool,
            tc.tile_pool(name="mem", bufs=16) as mpool,
            tc.tile_pool(name="dts", bufs=3) as dpool,
            tc.tile_pool(name="acts", bufs=2) as apool,
            tc.tile_pool(name="cp", bufs=4) as cppool,
            tc.tile_pool(name="small", bufs=1) as spool,
            tc.tile_pool(name="psz", bufs=1, space="PSUM") as pzpool,
            tc.tile_pool(name="pst", bufs=1, space="PSUM") as ptpool,
            tc.tile_pool(name="psb", bufs=6, space="PSUM") as pspool,
        ):
            # ---- constants, then W as two halves with tile (0,0) between
            c8_sb = spool.tile([128, 35], f8, tag="c8")
            nc.sync.dma_start(out=c8_sb[:], in_=c8[:])
            wt_a = wpool.tile([128, 16 * D_SH], f8, tag="wta")
            nc.sync.dma_start(out=wt_a[:], in_=wtpc[:, 0 : 16 * D_SH])
            mt00 = mpool.tile([128, NT], f8, tag="m", name="mt00")
            nc.sync.dma_start(out=mt00[:], in_=memT[0:128, 0:NT])
            wt_b = wpool.tile([128, 16 * D_SH], f8, tag="wtb")
            nc.sync.dma_start(out=wt_b[:], in_=wtpc[:, 16 * D_SH : 32 * D_SH])
            c32_sb = spool.tile([128, 5], f32, tag="c32")
            nc.scalar.dma_start(out=c32_sb[:], in_=c32[:])
            qc_sb = c8_sb[:, 0:32]
            ones_sb = c8_sb[:, 32:33]
            neg2_sb = c8_sb[:, 33:34]
            selsum8 = c8_sb[:, 34:35]
            b4 = c32_sb[:, 0:4]
            ones32_sb = c32_sb[:, 4:5]
            # activation table preload with no DMA dependency
            dum_in = spool.tile([1, 4], f32, tag="dumin")
            nc.vector.memset(dum_in[:], 0.25)
            dum = spool.tile([1, 4], f32, tag="dum")
            nc.scalar.activation(dum[:], dum_in[:], AF.Sigmoid)
            nc.scalar.activation(dum[:], dum_in[:], AF.Abs)

            zps = pzpool.tile([128, D_SH], f32, tag="z")
            nc.vector.memset(zps[:], 0.0)
            ztp = ptpool.tile([128, 512], f32, tag="ztp")
            zcp = spool.tile([128, D_SH], f16, tag="zcp")
            gpos = spool.tile([128, DC], f32, tag="gpos")

            # PE warmup: junk matmuls (zeroed SBUF, psum region reused by
            # strip-sums whose start=True clears it) keep HAM at K=8/8
            # through the W-completion so the gate runs at 2.4GHz.
            wjunk = spool.tile([128, 512], f8, tag="wjunk")
            nc.vector.memset(wjunk[:], 0.5)
            for _w in range(14):
                nc.tensor.matmul(
                    ztp[0:1, :], wjunk[:, 0:1], wjunk[:, 0:512],
                    start=(_w == 0), stop=(_w == 13), skip_group_check=True,
                )

            # ---- gate, per 128-d chunk ----
            for c in range(DC):
                for j in range(32):
                    r = j % 4
                    nc.tensor.matmul(
                        zps[32 * r : 32 * r + 1, c * 128 : (c + 1) * 128],
                        qc_sb[:, j : j + 1],
                        (wt_a if c < 2 else wt_b)[
                            :, (c % 2) * NT + j * 128 : (c % 2) * NT + (j + 1) * 128
                        ],
                        start=(j < 4),
                        stop=(j >= 28),
                        tile_position=(0, 32 * r),
                        skip_group_check=True,
                    )
                with tc.high_priority():
                    csl = slice(c * 128, (c + 1) * 128)
                    nc.scalar.activation(zcp[:, csl], zps[:, csl], AF.Copy)
                    # fused strip-sum + transpose
                    nc.tensor.matmul(
                        ztp[:, c : c + 1],
                        zcp[:, csl],
                        selsum8,
                        start=True,
                        stop=True,
                        skip_group_check=True,
                    )
                    # g_c = sigmoid(z_c + b_c) straight from psum
                    nc.scalar.activation(
                        gpos[:, c : c + 1], ztp[:, c : c + 1], AF.Sigmoid,
                        bias=b4[:, c : c + 1],
                    )

            # gate-sum correction, right after the gate
            with tc.high_priority():
                gsps = ztp[0:1, 4:8]
                nc.tensor.matmul(
                    gsps, ones32_sb, gpos[:], start=True, stop=True,
                    skip_group_check=True,
                )
                gs_sb = spool.tile([1, DC], f16, tag="gs")
                nc.scalar.activation(gs_sb[:], gsps, AF.Copy)

            out_dmas = [(outp[16:17, 0:DC], gs_sb[:])]

            # ---- main loop ----
            for k in range(NK):
                banks = [
                    pspool.tile([128, 512], f32, tag="bank", name=f"bank{k}a"),
                    pspool.tile([128, 512], f32, tag="bank", name=f"bank{k}b"),
                ]
                total = [0] * NG
                for c in range(DC):
                    npass = 1 if (k, c) in _ACT_TILES else 2
                    for j in range(NG):
                        total[j] += npass
                seen = [0] * NG
                mts = []
                for c in range(DC):
                    if k == 0 and c == 0:
                        mt = mt00
                    else:
                        mt = mpool.tile([128, NT], f8, tag="m")
                        nc.sync.dma_start(
                            out=mt[:],
                            in_=memT[c * 128 : (c + 1) * 128, k * NT : (k + 1) * NT],
                        )
                    mts.append(mt)
                    if (k, c) not in _ACT_TILES:
                        for j in range(NG):
                            nc.tensor.matmul(
                                banks[j // 4][32 * (j % 4) : 32 * (j % 4) + 1, :],
                                ones_sb,
                                mt[:, j * 512 : (j + 1) * 512],
                                start=(seen[j] == 0),
                                stop=(seen[j] == total[j] - 1),
                                tile_position=(0, 32 * (j % 4)),
                                skip_group_check=True,
                            )
                            seen[j] += 1
                for c in range(DC):
                    mt = mts[c]
                    if (k, c) in _ACT_TILES:
                        at = apool.tile([128, NT], f8, tag="a")
                        nc.scalar.activation(
                            at[:], mt[:], AF.Abs,
                            bias=gpos[:, c : c + 1], scale=-1.0,
                        )
                        src_, stat = at, ones_sb
                    else:
                        dt = dpool.tile([128, NT], f8, tag="d")
                        nc.vector.tensor_scalar(
                            dt[:], mt[:],
                            gpos[:, c : c + 1], 0.0,
                            A.subtract, A.min,
                        )
                        src_, stat = dt, neg2_sb
                    for j in range(NG):
                        nc.tensor.matmul(
                            banks[j // 4][32 * (j % 4) : 32 * (j % 4) + 1, :],
                            stat,
                            src_[:, j * 512 : (j + 1) * 512],
                            start=(seen[j] == 0),
                            stop=(seen[j] == total[j] - 1),
                            tile_position=(0, 32 * (j % 4)),
                            skip_group_check=True,
                        )
                        seen[j] += 1
                cp = cppool.tile([128, 1024], f16, tag="cp")
                for h in range(2):
                    csl = slice(512 * h, 512 * h + 512)
                    if k == 3 and h == 1:
                        nc.vector.tensor_copy(cp[:, csl], banks[h][:])
                    else:
                        nc.scalar.activation(cp[:, csl], banks[h][:], AF.Copy)
                out_dmas.append((outp[4 * k : 4 * k + 4, :], cp[0:128:32, :]))

            # ---- outputs: sync ring, after every input dma_start ----
            for dst, src in out_dmas:
                nc.sync.dma_start(out=dst, in_=src)

    nc.compile()
    return nc


def _get_nc():
    if "nc" not in _CACHE:
        _CACHE["nc"] = _build()
    return _CACHE["nc"]


def kernel(query, W, b, memory, _trace=False, _return_raw=False):
    f8 = ml_dtypes.float8_e3m4
    query = np.asarray(query, dtype=np.float32)
    W = np.asarray(W, dtype=np.float32)
    b = np.asarray(b, dtype=np.float32)
    memory = np.asarray(memory, dtype=np.float32)

    mem8T = np.ascontiguousarray(memory.astype(f8).T)       # [D, N] fp8
    W8 = W.astype(f8)
    q8 = query.reshape(32, 128).astype(f8).T                # [128, 32]
    c8 = np.zeros((128, 35), dtype=f8)
    c8[:, 0:32] = q8
    c8[:, 32] = f8(1.0)
    c8[:, 33] = f8(-2.0)
    c8[0:128:32, 34] = f8(1.0)

    in_maps = []
    for c in range(N_CORES):
        sl = slice(c * D_SH, (c + 1) * D_SH)
        wsh = W8[sl, :]                       # [512, 4096]
        # wtpc[p, cc*4096 + j*128 + n'] = wsh[cc*128 + n', j*128 + p]
        wtpc = np.ascontiguousarray(
            wsh.reshape(4, 128, 32, 128).transpose(3, 0, 2, 1).reshape(128, -1)
        )
        c32 = np.zeros((128, 5), dtype=np.float32)
        c32[:, 0:4] = b[sl].reshape(4, 128).T
        c32[:, 4] = 1.0
        in_maps.append(
            {
                "memT": np.ascontiguousarray(mem8T[sl, :]),
                "wtpc": wtpc,
                "c8": c8,
                "c32": c32,
            }
        )

    nc = _get_nc()
    res = run_bass_kernel_spmd(
        nc, in_maps, list(range(N_CORES)), trace=_trace
    )

    total = np.zeros(N, dtype=np.float64)
    for c in range(N_CORES):
        out = res.results[c]["outp"].astype(np.float64)
        gsum = out[16, 0:DC]                  # sum of g per d-chunk
        # row 4k+i, col 512h+n  ->  block k, group j=4h+i
        rows = out[0:16].reshape(NK, 4, 2, 512).transpose(0, 2, 1, 3)
        rows = np.ascontiguousarray(rows).reshape(NK, NG, 512)
        corr = np.array(
            [
                [sum(gsum[ci] for ci in _dve_chunks(k, j)) for j in range(NG)]
                for k in range(NK)
            ]
        )
        total += (rows - corr[:, :, None]).reshape(N)
    sims = (1.0 - total / D).astype(np.float32)
    mask = sims >= THRESHOLD
    if _return_raw:
        return (sims, mask), res
    return sims, mask
